# revision 21
# baseline (speedup 1.0000x reference)
"""Two-layer GATv2 on 8 Trainium2 NeuronCores.

Strategy (edge-parallel, dst-sharded):
  * Self-loops appended, edges sorted by dst, sharded so core k owns dst nodes
    [k*6250, (k+1)*6250).  Each core's nodes are packed into "superchunks" of
    120 consecutive dst nodes; each superchunk's edges are padded to fixed
    chunk counts and split into src<32768 / src>=32768 regions (dma_gather has
    int16 indices).
  * Per edge, xl[src] / xr[dst] rows (fp16, 256B) are fetched with dma_gather.
    Segment softmax uses the unshifted form exp(logit - 4) (every node has a
    self-loop so denominators are well-conditioned; logits are O(+-8)).
  * Per-128-edge chunk, a one-hot(slot) matrix is built on-chip and the
    segment sum (numerator and denominator together) is computed as a matmul
    accumulated in PSUM across the superchunk.
  * Two SPMD launches: layer 1 -> h1 shards -> host gather -> layer 2 +
    log_softmax.  Weights are replicated; node transforms run on-device.
"""

import sys

sys.path.insert(0, "/opt/trn_rl_repo")

from contextlib import ExitStack

import numpy as np

import concourse.bass as bass
import concourse.tile as tile
from concourse import bacc, mybir
from concourse.bass_utils import run_bass_kernel_spmd

# ---- problem constants (hardcoded from the GAT spec) ----
N, DIN, HEADS, DH, DOUT = 50000, 128, 8, 16, 40
HD = HEADS * DH  # 128
NEG = 0.2
NCORES = 8
EXP_SHIFT = -4.0


def _configure(n):
    """Set graph-size-derived geometry (module globals).  Default n=50000."""
    global N, NPC, NODES_SC, TRASH, NSC, G, NG, NSCP, LO, NPAD, LOCROWS
    N = n
    NPC = N // NCORES  # dst nodes per core
    NODES_SC = 120  # dst nodes per superchunk
    TRASH = 120  # slot for padding edges
    NSC = -(-NPC // NODES_SC)  # superchunks per core
    G = 8  # superchunks per gather group
    NG = -(-NSC // G)
    NSCP = NG * G
    LO = min(32768, ((N + 127) // 128) * 128)  # int16-safe table split
    # local (dst) table: covers [A, A + 128-misalignment + NSC*120)
    LOCROWS = ((128 + NSC * NODES_SC + 127) // 128) * 128
    max_a = ((NCORES - 1) * NPC) // 128 * 128
    NPAD = max(((N + 127) // 128) * 128 + 128, max_a + LOCROWS)  # padded nodes


_configure(50000)
assert NPAD == 50176 and LOCROWS == 6528 and NSC == 53 and NG == 7

F16 = mybir.dt.float16
F32 = mybir.dt.float32
I16 = mybir.dt.int16
ALU = None  # set after mybir import


def _round128(v):
    return max(128, ((int(v) + 127) // 128) * 128)


def _wrap_idx(flat):
    """dma_gather index layout: idx j -> [j%16, j//16], replicated to 128 rows."""
    a = np.asarray(flat, np.int16).reshape(-1, 16).T  # [16, W]
    return np.tile(a, (8, 1))  # [128, W]


def _preprocess(edge_index):
    """Sort/shard/pad the edge list.  Returns (cfg, per-core dict of arrays)."""
    src = np.concatenate([edge_index[0], np.arange(N, dtype=np.int64)])
    dst = np.concatenate([edge_index[1], np.arange(N, dtype=np.int64)])
    order = np.argsort(dst, kind="stable")
    src = src[order].astype(np.int32)
    dst = dst[order].astype(np.int32)

    core_edges = []  # per core: list over sc of (lo_src, lo_dst, hi_src, hi_dst)
    max_lo = 1
    max_hi = 1
    for k in range(NCORES):
        a = np.searchsorted(dst, k * NPC, side="left")
        b = np.searchsorted(dst, (k + 1) * NPC, side="left")
        s, d = src[a:b], dst[a:b]
        bounds = k * NPC + np.arange(NSC + 1) * NODES_SC
        ofs = np.searchsorted(d, bounds, side="left")
        scs = []
        for t in range(NSC):
            ss = s[ofs[t] : ofs[t + 1]]
            dd = d[ofs[t] : ofs[t + 1]]
            m = ss < LO
            lo_s, lo_d = ss[m], dd[m]
            hi_s, hi_d = ss[~m], dd[~m]
            max_lo = max(max_lo, len(lo_s))
            max_hi = max(max_hi, len(hi_s))
            scs.append((lo_s, lo_d, hi_s, hi_d))
        core_edges.append(scs)

    CLO = _round128(max_lo)
    CHI = _round128(max_hi)
    CLOC, CHIC = CLO // 128, CHI // 128
    CCH = CLOC + CHIC
    GC = G * CCH
    cfg = dict(CLO=CLO, CHI=CHI, CLOC=CLOC, CHIC=CHIC, CCH=CCH, GC=GC)

    cores = []
    for k in range(NCORES):
        A = (k * NPC) // 128 * 128  # local-table base (128-aligned)
        idxlo = np.zeros((NG, 128, G * CLO // 16), np.int16)
        idxhi = np.zeros((NG, 128, G * CHI // 16), np.int16)
        idxxr = np.zeros((NG, 128, G * (CLO + CHI) // 16), np.int16)
        dstloc = np.full((NG, 128, GC), TRASH, np.float16)
        for g in range(NG):
            flat_lo_src = np.zeros(G * CLO, np.int32)
            flat_hi_src = np.zeros(G * CHI, np.int32)
            flat_lo_xr = np.zeros(G * CLO, np.int32)
            flat_hi_xr = np.zeros(G * CHI, np.int32)
            flat_lo_slot = np.full(G * CLO, TRASH, np.int32)
            flat_hi_slot = np.full(G * CHI, TRASH, np.int32)
            for j in range(G):
                t = g * G + j
                if t >= NSC:
                    continue
                lo_s, lo_d, hi_s, hi_d = core_edges[k][t]
                base = k * NPC + t * NODES_SC
                nl, nh = len(lo_s), len(hi_s)
                flat_lo_src[j * CLO : j * CLO + nl] = lo_s
                flat_lo_xr[j * CLO : j * CLO + nl] = lo_d - A
                flat_lo_slot[j * CLO : j * CLO + nl] = lo_d - base
                flat_hi_src[j * CHI : j * CHI + nh] = hi_s - LO
                flat_hi_xr[j * CHI : j * CHI + nh] = hi_d - A
                flat_hi_slot[j * CHI : j * CHI + nh] = hi_d - base
            idxlo[g] = _wrap_idx(flat_lo_src)
            idxhi[g] = _wrap_idx(flat_hi_src)
            idxxr[g] = _wrap_idx(np.concatenate([flat_lo_xr, flat_hi_xr]))
            slots = np.concatenate([flat_lo_slot, flat_hi_slot])
            dstloc[g] = slots.reshape(GC, 128).T.astype(np.float16)
        cores.append(dict(idxlo=idxlo, idxhi=idxhi, idxxr=idxxr, dstloc=dstloc, A=A))
    return cfg, cores


def _emit_transform(tc, ctx, xT, xTl, w_t, bias_t, tab, tab_loc):
    """tab[n] = x[n] @ W + bias (node-major, fp16), for the global and local tables."""
    nc = tc.nc
    ppool = ctx.enter_context(tc.tile_pool(name="tpsum", bufs=4, space="PSUM"))
    spool = ctx.enter_context(tc.tile_pool(name="tout", bufs=4))
    cpool = ctx.enter_context(tc.tile_pool(name="tconst", bufs=1))
    ones1 = cpool.tile([1, 128], F16)
    nc.vector.memset(ones1[:], 1.0)
    for src_x, dst_tab, n_t in ((xT, tab, NPAD // 128), (xTl, tab_loc, LOCROWS // 128)):
        for t in range(n_t):
            ps = ppool.tile([128, 128], F32, tag="tps")
            nc.tensor.matmul(
                out=ps[:],
                lhsT=src_x[:, t * 128 : (t + 1) * 128],
                rhs=w_t[:],
                start=True,
                stop=False,
            )
            nc.tensor.matmul(
                out=ps[:], lhsT=ones1[:], rhs=bias_t[:], start=False, stop=True
            )
            ot = spool.tile([128, 128], F16, tag="tob")
            nc.scalar.copy(ot[:], ps[:])
            nc.sync.dma_start(dst_tab[t * 128 : (t + 1) * 128, :], ot[:])


def _emit_edge_phase(tc, ctx, cfg, io, layer, accpool):
    """Gather + edge compute + segment-softmax aggregation. layer in (1, 2)."""
    nc = tc.nc
    add, mult, mx, mn = (
        mybir.AluOpType.add,
        mybir.AluOpType.mult,
        mybir.AluOpType.max,
        mybir.AluOpType.min,
    )
    CLOC, CHIC, CCH, GC = cfg["CLOC"], cfg["CHIC"], cfg["CCH"], cfg["GC"]
    CLO, CHI = cfg["CLO"], cfg["CHI"]
    NF = HD if layer == 1 else DOUT  # feature count used by this layer
    NH = HEADS if layer == 1 else 1
    DHH = DH if layer == 1 else DOUT
    MCOL = NF + NH  # psum columns: features + per-head denominators

    const = ctx.enter_context(tc.tile_pool(name="econst", bufs=1))
    gpool = ctx.enter_context(tc.tile_pool(name="gath", bufs=1))
    opool = ctx.enter_context(tc.tile_pool(name="onehot", bufs=2))
    ipool = ctx.enter_context(tc.tile_pool(name="idx", bufs=2))
    wpool = ctx.enter_context(tc.tile_pool(name="wexp", bufs=2))
    psum = ctx.enter_context(tc.tile_pool(name="aggp", bufs=3, space="PSUM"))

    attr = const.tile([128, 128], F16)
    nc.sync.dma_start(attr[:], io["attr"].ap())
    iot = const.tile([128, 128], F16)
    nc.sync.dma_start(iot[:], io["iota"].ap())
    shift = const.tile([128, 1], F32)
    nc.vector.memset(shift[:], EXP_SHIFT)

    acc = accpool.tile([128, NSCP * MCOL], F32, tag="acc")
    xlt = gpool.tile([128, GC * 128], F16, tag="xl")
    xrt = gpool.tile([128, GC * 128], F16, tag="xr")
    xlt3 = xlt[:].rearrange("p (c e) -> p c e", e=128)
    xrt3 = xrt[:].rearrange("p (c e) -> p c e", e=128)

    for g in range(NG):
        ilo = ipool.tile([128, G * CLO // 16], I16, tag="ilo")
        nc.sync.dma_start(ilo[:], io["idxlo"].ap()[g])
        ihi = ipool.tile([128, G * CHI // 16], I16, tag="ihi")
        nc.sync.dma_start(ihi[:], io["idxhi"].ap()[g])
        ixr = ipool.tile([128, G * (CLO + CHI) // 16], I16, tag="ixr")
        nc.sync.dma_start(ixr[:], io["idxxr"].ap()[g])
        dl = ipool.tile([128, GC], F16, tag="dl")
        nc.sync.dma_start(dl[:], io["dstloc"].ap()[g])

        # dma_gather is limited to 1024 indices per call on this runtime;
        # split into 8-chunk calls round-robined over the 4 SWDGE queues.
        qrr = [0]

        def _gathers(out3, chunk0, nchunks, tab_ap, idxs, srows):
            for c0 in range(0, nchunks, 8):
                nch = min(8, nchunks - c0)
                nc.gpsimd.dma_gather(
                    out_ap=out3[:, chunk0 + c0 : chunk0 + c0 + nch, :],
                    in_ap=tab_ap[srows, :],
                    idxs_ap=idxs[:, c0 * 8 : (c0 + nch) * 8],
                    num_idxs=nch * 128,
                    num_idxs_reg=nch * 128,
                    elem_size=128,
                    queue_num=qrr[0] % 4,
                )
                qrr[0] += 1

        _gathers(xlt3, 0, G * CLOC, io["xl_tab"].ap(), ilo[:], slice(0, LO))
        _gathers(xlt3, G * CLOC, G * CHIC, io["xl_tab"].ap(), ihi[:], slice(LO, NPAD))
        _gathers(xrt3, 0, GC, io["xr_tab"].ap(), ixr[:], slice(0, LOCROWS))

        # U0 = xl + xr ; U = lrelu(U0) ; T = U * att   (all in-place over xrt)
        if NF == 128:
            xl_nf, xr_nf = xlt[:], xrt[:]
        else:
            xl_nf = xlt3[:, :, 0:NF]
            xr_nf = xrt3[:, :, 0:NF]
        nc.vector.tensor_tensor(out=xr_nf, in0=xl_nf, in1=xr_nf, op=add)
        nc.vector.scalar_tensor_tensor(
            out=xr_nf, in0=xr_nf, scalar=NEG, op0=mult, in1=xr_nf, op1=mx
        )
        att_b = attr[:].rearrange("p f -> p () f")[:, :, 0:NF].to_broadcast(
            [128, GC, NF]
        )
        nc.vector.tensor_tensor(out=xr_nf, in0=xr_nf, in1=att_b, op=mult)

        lg = wpool.tile([128, GC * NH], F32, tag="lg")
        red_in = (
            xrt[:].rearrange("p (c h d) -> p c h d", h=NH, d=DH)
            if layer == 1
            else xrt3[:, :, 0:NF].rearrange("p c d -> p c () d")
        )
        nc.vector.tensor_reduce(
            out=lg[:], in_=red_in, axis=mybir.AxisListType.X, op=add
        )
        wt = wpool.tile([128, GC * NH], F16, tag="wt")
        nc.scalar.activation(
            out=wt[:], in_=lg[:], func=mybir.ActivationFunctionType.Exp,
            bias=shift[:], scale=1.0,
        )
        # wf = w * xl (in-place over xlt's used columns)
        w_b = (
            wt[:]
            .rearrange("p (c h) -> p c h ()", h=NH)
            .to_broadcast([128, GC, NH, DHH])
        )
        wf_out = (
            xlt[:].rearrange("p (c h d) -> p c h d", h=NH, d=DH)
            if layer == 1
            else xlt3[:, :, 0:NF].rearrange("p c d -> p c () d")
        )
        nc.vector.tensor_tensor(out=wf_out, in0=wf_out, in1=w_b, op=mult)

        # one-hot built per pair of superchunks, then aggregation matmuls.
        # the one-hot covers a pair's chunks in chunk-layout order: the lo
        # chunks of both scs (contiguous) and the hi chunks of both.
        wt3 = wt[:].rearrange("p (c h) -> p c h", h=NH)
        for jp in range(G // 2):
            j0 = jp * 2
            oh_lo = opool.tile([128, 2 * CLOC * 128], F16, tag="ohlo")
            dl_b = (
                dl[:, j0 * CLOC : (j0 + 2) * CLOC]
                .rearrange("p c -> p c ()")
                .to_broadcast([128, 2 * CLOC, 128])
            )
            iot_b = iot[:].rearrange("p f -> p () f").to_broadcast([128, 2 * CLOC, 128])
            nc.vector.tensor_tensor(
                out=oh_lo[:].rearrange("p (c e) -> p c e", e=128),
                in0=dl_b, in1=iot_b, op=mybir.AluOpType.is_equal,
            )
            oh_hi = opool.tile([128, 2 * CHIC * 128], F16, tag="ohhi")
            dl_bh = (
                dl[:, G * CLOC + j0 * CHIC : G * CLOC + (j0 + 2) * CHIC]
                .rearrange("p c -> p c ()")
                .to_broadcast([128, 2 * CHIC, 128])
            )
            iot_bh = iot[:].rearrange("p f -> p () f").to_broadcast([128, 2 * CHIC, 128])
            nc.vector.tensor_tensor(
                out=oh_hi[:].rearrange("p (c e) -> p c e", e=128),
                in0=dl_bh, in1=iot_bh, op=mybir.AluOpType.is_equal,
            )
            oh_lo3 = oh_lo[:].rearrange("p (c e) -> p c e", e=128)
            oh_hi3 = oh_hi[:].rearrange("p (c e) -> p c e", e=128)
            for dj in range(2):
                j = j0 + dj
                scg = g * G + j
                ps_n = psum.tile([128, NF], F32, tag="aggn")
                ps_d = psum.tile([128, NH], F32, tag="aggd")
                chunks = [(j * CLOC + c, oh_lo3[:, dj * CLOC + c, :]) for c in range(CLOC)] + [
                    (G * CLOC + j * CHIC + c, oh_hi3[:, dj * CHIC + c, :])
                    for c in range(CHIC)
                ]
                nch = len(chunks)
                for ci, (c, oh_ap) in enumerate(chunks):
                    first, last = ci == 0, ci == nch - 1
                    nc.tensor.matmul(
                        out=ps_n[:], lhsT=oh_ap, rhs=xlt3[:, c, 0:NF],
                        start=first, stop=last,
                    )
                    nc.tensor.matmul(
                        out=ps_d[:], lhsT=oh_ap, rhs=wt3[:, c, :],
                        start=first, stop=last,
                    )
                nc.scalar.copy(acc[:, scg * MCOL : scg * MCOL + NF], ps_n[:])
                nc.scalar.copy(acc[:, scg * MCOL + NF : (scg + 1) * MCOL], ps_d[:])
    return acc


def _build_layer1(cfg):
    nc = bacc.Bacc("TRN2", target_bir_lowering=False, debug=False, num_devices=NCORES, num_swdge_queues=4)
    CLO, CHI, GC = cfg["CLO"], cfg["CHI"], cfg["GC"]
    io = {
        "xT": nc.dram_tensor("xT", [128, NPAD], F16, kind="ExternalInput"),
        "xTloc": nc.dram_tensor("xTloc", [128, LOCROWS], F16, kind="ExternalInput"),
        "wl": nc.dram_tensor("wl", [128, 128], F16, kind="ExternalInput"),
        "wr": nc.dram_tensor("wr", [128, 128], F16, kind="ExternalInput"),
        "bias": nc.dram_tensor("bias", [1, 128], F16, kind="ExternalInput"),
        "attr": nc.dram_tensor("attr", [128, 128], F16, kind="ExternalInput"),
        "iota": nc.dram_tensor("iota", [128, 128], F16, kind="ExternalInput"),
        "idxlo": nc.dram_tensor("idxlo", [NG, 128, G * CLO // 16], I16, kind="ExternalInput"),
        "idxhi": nc.dram_tensor("idxhi", [NG, 128, G * CHI // 16], I16, kind="ExternalInput"),
        "idxxr": nc.dram_tensor("idxxr", [NG, 128, G * (CLO + CHI) // 16], I16, kind="ExternalInput"),
        "dstloc": nc.dram_tensor("dstloc", [NG, 128, GC], F16, kind="ExternalInput"),
        "xl_tab": nc.dram_tensor("xl_tab", [NPAD, 128], F16),
        "xr_tab": nc.dram_tensor("xr_tab", [LOCROWS, 128], F16),
        "h1s": nc.dram_tensor("h1s", [NSCP * NODES_SC, HD], F32, kind="ExternalOutput"),
    }
    MCOL = HD + HEADS
    with tile.TileContext(nc) as tc:
        with ExitStack() as ctx:
            # transforms (own pools, freed before edge phase)
            with ExitStack() as tctx:
                xpool = tctx.enter_context(tc.tile_pool(name="xt", bufs=1))
                wpool = tctx.enter_context(tc.tile_pool(name="wz", bufs=1))
                xT = xpool.tile([128, NPAD], F16)
                nc.sync.dma_start(xT[:], io["xT"].ap())
                xTl = xpool.tile([128, LOCROWS], F16)
                nc.sync.dma_start(xTl[:], io["xTloc"].ap())
                wl = wpool.tile([128, 128], F16)
                nc.sync.dma_start(wl[:], io["wl"].ap())
                wr = wpool.tile([128, 128], F16)
                nc.sync.dma_start(wr[:], io["wr"].ap())
                bt = wpool.tile([1, 128], F16)
                nc.sync.dma_start(bt[:], io["bias"].ap())
                zt = wpool.tile([1, 128], F16)
                nc.vector.memset(zt[:], 0.0)
                ppool = tctx.enter_context(tc.tile_pool(name="tpsum", bufs=4, space="PSUM"))
                spool = tctx.enter_context(tc.tile_pool(name="tout", bufs=4))
                ones1 = wpool.tile([1, 128], F16)
                nc.vector.memset(ones1[:], 1.0)
                for t in range(NPAD // 128):
                    ps = ppool.tile([128, 128], F32, tag="tps")
                    nc.tensor.matmul(out=ps[:], lhsT=xT[:, t * 128 : (t + 1) * 128], rhs=wl[:], start=True, stop=False)
                    nc.tensor.matmul(out=ps[:], lhsT=ones1[:], rhs=bt[:], start=False, stop=True)
                    ot = spool.tile([128, 128], F16, tag="tob")
                    nc.scalar.copy(ot[:], ps[:])
                    nc.sync.dma_start(io["xl_tab"].ap()[t * 128 : (t + 1) * 128, :], ot[:])
                for t in range(LOCROWS // 128):
                    ps = ppool.tile([128, 128], F32, tag="tps")
                    nc.tensor.matmul(out=ps[:], lhsT=xTl[:, t * 128 : (t + 1) * 128], rhs=wr[:], start=True, stop=True)
                    ot = spool.tile([128, 128], F16, tag="tob")
                    nc.scalar.copy(ot[:], ps[:])
                    nc.sync.dma_start(io["xr_tab"].ap()[t * 128 : (t + 1) * 128, :], ot[:])

            accpool = ctx.enter_context(tc.tile_pool(name="hacc", bufs=1))
            with ExitStack() as ectx:
                acc = _emit_edge_phase(tc, ectx, cfg, io, layer=1, accpool=accpool)
            # post: h1 = elu(num / den); write shards
            hpool2 = ctx.enter_context(tc.tile_pool(name="post", bufs=1))
            acc3 = acc[:].rearrange("p (s q) -> p s q", q=MCOL)
            dnv = acc3[:, :, HD:MCOL]
            nc.vector.tensor_scalar_add(dnv, dnv, 1e-16)
            rc = hpool2.tile([128, NSCP * HEADS], F32, tag="rc")
            nc.vector.reciprocal(
                out=rc[:].rearrange("p (s h) -> p s h", h=HEADS),
                in_=dnv,
            )
            h1o = hpool2.tile([128, NSCP * HD], F32, tag="h1o")
            h1o4 = h1o[:].rearrange("p (s h d) -> p s h d", h=HEADS, d=DH)
            rc_b = (
                rc[:]
                .rearrange("p (s h) -> p s h ()", h=HEADS)
                .to_broadcast([128, NSCP, HEADS, DH])
            )
            num4 = acc3[:, :, 0:HD].rearrange("p s (h d) -> p s h d", h=HEADS)
            nc.vector.tensor_tensor(out=h1o4, in0=num4, in1=rc_b, op=mybir.AluOpType.mult)
            tmp = hpool2.tile([128, NSCP * HD], F32, tag="tmp")
            nc.vector.tensor_scalar_min(tmp[:], h1o[:], 0.0)
            nc.scalar.activation(out=tmp[:], in_=tmp[:], func=mybir.ActivationFunctionType.Exp)
            nc.vector.tensor_scalar_add(tmp[:], tmp[:], -1.0)
            nc.vector.scalar_tensor_tensor(
                out=h1o[:], in0=h1o[:], scalar=0.0, op0=mybir.AluOpType.max,
                in1=tmp[:], op1=mybir.AluOpType.add,
            )
            for scg in range(NSCP):
                nc.sync.dma_start(
                    io["h1s"].ap()[scg * NODES_SC : (scg + 1) * NODES_SC, :],
                    h1o[0:NODES_SC, scg * HD : (scg + 1) * HD],
                )
    nc.compile()
    return nc


def _build_layer2(cfg):
    nc = bacc.Bacc("TRN2", target_bir_lowering=False, debug=False, num_devices=NCORES, num_swdge_queues=4)
    CLO, CHI, GC = cfg["CLO"], cfg["CHI"], cfg["GC"]
    io = {
        "xT": nc.dram_tensor("xT", [128, NPAD], F16, kind="ExternalInput"),
        "xTloc": nc.dram_tensor("xTloc", [128, LOCROWS], F16, kind="ExternalInput"),
        "wl": nc.dram_tensor("wl", [128, 128], F16, kind="ExternalInput"),  # [Wl2|0]
        "wr": nc.dram_tensor("wr", [128, 128], F16, kind="ExternalInput"),  # [Wr2|0]
        "bias": nc.dram_tensor("bias", [1, 128], F16, kind="ExternalInput"),  # [b2|0]
        "attr": nc.dram_tensor("attr", [128, 128], F16, kind="ExternalInput"),
        "iota": nc.dram_tensor("iota", [128, 128], F16, kind="ExternalInput"),
        "idxlo": nc.dram_tensor("idxlo", [NG, 128, G * CLO // 16], I16, kind="ExternalInput"),
        "idxhi": nc.dram_tensor("idxhi", [NG, 128, G * CHI // 16], I16, kind="ExternalInput"),
        "idxxr": nc.dram_tensor("idxxr", [NG, 128, G * (CLO + CHI) // 16], I16, kind="ExternalInput"),
        "dstloc": nc.dram_tensor("dstloc", [NG, 128, GC], F16, kind="ExternalInput"),
        "xl_tab": nc.dram_tensor("xl_tab", [NPAD, 128], F16),
        "xr_tab": nc.dram_tensor("xr_tab", [LOCROWS, 128], F16),
        "outs": nc.dram_tensor("outs", [NSCP * NODES_SC, DOUT], F32, kind="ExternalOutput"),
    }
    MCOL = DOUT + 1
    with tile.TileContext(nc) as tc:
        with ExitStack() as ctx:
            with ExitStack() as tctx:
                xpool = tctx.enter_context(tc.tile_pool(name="xt", bufs=1))
                wpool = tctx.enter_context(tc.tile_pool(name="wz", bufs=1))
                xT = xpool.tile([128, NPAD], F16)
                nc.sync.dma_start(xT[:], io["xT"].ap())
                xTl = xpool.tile([128, LOCROWS], F16)
                nc.sync.dma_start(xTl[:], io["xTloc"].ap())
                wl = wpool.tile([128, 128], F16)
                nc.sync.dma_start(wl[:], io["wl"].ap())
                wr = wpool.tile([128, 128], F16)
                nc.sync.dma_start(wr[:], io["wr"].ap())
                bt = wpool.tile([1, 128], F16)
                nc.sync.dma_start(bt[:], io["bias"].ap())
                ones1 = wpool.tile([1, 128], F16)
                nc.vector.memset(ones1[:], 1.0)
                ppool = tctx.enter_context(tc.tile_pool(name="tpsum", bufs=4, space="PSUM"))
                spool = tctx.enter_context(tc.tile_pool(name="tout", bufs=4))
                for t in range(NPAD // 128):
                    ps = ppool.tile([128, 128], F32, tag="tps")
                    nc.tensor.matmul(out=ps[:], lhsT=xT[:, t * 128 : (t + 1) * 128], rhs=wl[:], start=True, stop=False)
                    nc.tensor.matmul(out=ps[:], lhsT=ones1[:], rhs=bt[:], start=False, stop=True)
                    ot = spool.tile([128, 128], F16, tag="tob")
                    nc.scalar.copy(ot[:], ps[:])
                    nc.sync.dma_start(io["xl_tab"].ap()[t * 128 : (t + 1) * 128, :], ot[:])
                for t in range(LOCROWS // 128):
                    ps = ppool.tile([128, 128], F32, tag="tps")
                    nc.tensor.matmul(out=ps[:], lhsT=xTl[:, t * 128 : (t + 1) * 128], rhs=wr[:], start=True, stop=True)
                    ot = spool.tile([128, 128], F16, tag="tob")
                    nc.scalar.copy(ot[:], ps[:])
                    nc.sync.dma_start(io["xr_tab"].ap()[t * 128 : (t + 1) * 128, :], ot[:])

            accpool = ctx.enter_context(tc.tile_pool(name="hacc", bufs=1))
            with ExitStack() as ectx:
                acc = _emit_edge_phase(tc, ectx, cfg, io, layer=2, accpool=accpool)
            # post: out = log_softmax(num / den)
            hpool2 = ctx.enter_context(tc.tile_pool(name="post", bufs=1))
            acc3 = acc[:].rearrange("p (s q) -> p s q", q=MCOL)
            dnv = acc3[:, :, DOUT:MCOL]
            nc.vector.tensor_scalar_add(dnv, dnv, 1e-16)
            rc = hpool2.tile([128, NSCP], F32, tag="rc")
            nc.vector.reciprocal(out=rc[:].rearrange("p s -> p s ()"), in_=dnv)
            o2 = hpool2.tile([128, NSCP * DOUT], F32, tag="o2")
            o23 = o2[:].rearrange("p (s d) -> p s d", d=DOUT)
            rc_b = rc[:].rearrange("p s -> p s ()").to_broadcast([128, NSCP, DOUT])
            nc.vector.tensor_tensor(out=o23, in0=acc3[:, :, 0:DOUT], in1=rc_b, op=mybir.AluOpType.mult)
            ex = hpool2.tile([128, NSCP * DOUT], F32, tag="ex")
            nc.scalar.activation(out=ex[:], in_=o2[:], func=mybir.ActivationFunctionType.Exp)
            s2 = hpool2.tile([128, NSCP], F32, tag="s2")
            nc.vector.tensor_reduce(
                out=s2[:],
                in_=ex[:].rearrange("p (s d) -> p s d", d=DOUT),
                axis=mybir.AxisListType.X,
                op=mybir.AluOpType.add,
            )
            ln2 = hpool2.tile([128, NSCP], F32, tag="ln2")
            nc.scalar.activation(out=ln2[:], in_=s2[:], func=mybir.ActivationFunctionType.Ln)
            ln_b = ln2[:].rearrange("p s -> p s ()").to_broadcast([128, NSCP, DOUT])
            nc.vector.tensor_tensor(out=o23, in0=o23, in1=ln_b, op=mybir.AluOpType.subtract)
            for scg in range(NSCP):
                nc.sync.dma_start(
                    io["outs"].ap()[scg * NODES_SC : (scg + 1) * NODES_SC, :],
                    o2[0:NODES_SC, scg * DOUT : (scg + 1) * DOUT],
                )
    nc.compile()
    return nc


def _iota_tile():
    return np.tile(np.arange(128, dtype=np.float16), (128, 1))


def _inputs_layer1(cfg, cores, x, Wl1, Wr1, att1, b1):
    iota = _iota_tile()
    attr1 = np.zeros((1, 128), np.float32)
    attr1[0, :HD] = np.asarray(att1, np.float32).reshape(-1)
    attr1 = np.tile(attr1, (128, 1)).astype(np.float16)
    xpad = np.zeros((NPAD, DIN), np.float32)
    xpad[:N] = np.asarray(x, np.float32)
    xT = xpad.T.astype(np.float16).copy()
    in_maps = []
    for k in range(NCORES):
        A = cores[k]["A"]
        in_maps.append(
            {
                "xT": xT,
                "xTloc": xT[:, A : A + LOCROWS].copy(),
                "wl": np.asarray(Wl1, np.float32).astype(np.float16),
                "wr": np.asarray(Wr1, np.float32).astype(np.float16),
                "bias": np.asarray(b1, np.float32).reshape(1, HD).astype(np.float16),
                "attr": attr1,
                "iota": iota,
                "idxlo": cores[k]["idxlo"],
                "idxhi": cores[k]["idxhi"],
                "idxxr": cores[k]["idxxr"],
                "dstloc": cores[k]["dstloc"],
            }
        )
    return in_maps


def _inputs_layer2(cfg, cores, h1, Wl2, Wr2, att2, b2):
    iota = _iota_tile()
    a2 = np.zeros((1, 128), np.float32)
    a2[0, :DOUT] = np.asarray(att2, np.float32).reshape(-1)
    attr2 = np.tile(a2, (128, 1)).astype(np.float16)
    h1T = h1.T.astype(np.float16).copy()
    wl2 = np.zeros((128, 128), np.float32)
    wl2[:, :DOUT] = np.asarray(Wl2, np.float32)
    wr2 = np.zeros((128, 128), np.float32)
    wr2[:, :DOUT] = np.asarray(Wr2, np.float32)
    b2c = np.zeros((1, 128), np.float32)
    b2c[0, :DOUT] = np.asarray(b2, np.float32).reshape(-1)
    in_maps2 = []
    for k in range(NCORES):
        A = cores[k]["A"]
        in_maps2.append(
            {
                "xT": h1T,
                "xTloc": h1T[:, A : A + LOCROWS].copy(),
                "wl": wl2.astype(np.float16),
                "wr": wr2.astype(np.float16),
                "bias": b2c.astype(np.float16),
                "attr": attr2,
                "iota": iota,
                "idxlo": cores[k]["idxlo"],
                "idxhi": cores[k]["idxhi"],
                "idxxr": cores[k]["idxxr"],
                "dstloc": cores[k]["dstloc"],
            }
        )
    return in_maps2


def kernel(x, edge_index, Wl1, Wr1, att1, b1, Wl2, Wr2, att2, b2):
    x = np.asarray(x, np.float32)
    edge_index = np.asarray(edge_index)
    cfg, cores = _preprocess(edge_index)

    nc1 = _build_layer1(cfg)
    in_maps = _inputs_layer1(cfg, cores, x, Wl1, Wr1, att1, b1)
    res1 = run_bass_kernel_spmd(nc1, in_maps, core_ids=list(range(NCORES)))
    h1 = np.zeros((NPAD, HD), np.float32)
    for k in range(NCORES):
        h1[k * NPC : (k + 1) * NPC] = res1.results[k]["h1s"][:NPC]

    nc2 = _build_layer2(cfg)
    in_maps2 = _inputs_layer2(cfg, cores, h1, Wl2, Wr2, att2, b2)
    res2 = run_bass_kernel_spmd(nc2, in_maps2, core_ids=list(range(NCORES)))
    out = np.zeros((N, DOUT), np.float32)
    for k in range(NCORES):
        out[k * NPC : (k + 1) * NPC] = res2.results[k]["outs"][:NPC]
    return out
